# revision 47
# baseline (speedup 1.0000x reference)
"""Trainium2 Bass kernel for nn_Attention_29738353557815.

8-way tensor-parallel over heads:
  - core c owns q-heads {2c, 2c+1} and kv-head c//2 (k/v proj duplicated per
    core pair); projections run weights-stationary off a host-pretransposed
    hidden^T in fp16, producing q/k in [head_dim, T] fp16 layout
  - rms-norm folded into ln/exp on ACT; rope tables (cos/sin * sqrt(scale))
    host-precomputed in [hd, T] layout; rotate-half via half-tile
    tensor_tensor ops against a half-swapped sin table
  - attention in S^T layout ([key, query] tiles), column-narrowed per tile to
    the valid [c0, c1) query range implied by causality and the (sorted)
    segment ids; exp has bias=-4 so fp16 probabilities cannot overflow
    (cancels between numerator and row-sum); softmax denominator via
    ones-matmul column sums; normalization and sigmoid gating fused into one
    multiply before the o-projection
  - phase order k/v/q0/g0 -> attn h0 -> AllToAll 0 -> q1/g1 -> attn h1 ->
    AllToAll 1 -> o-proj, so each 28us collective overlaps the other head's
    compute; o-proj consumes h0 blocks first so it starts right after coll0
  - o-proj writes psum accumulators straight to DRAM (2 KiB runs)

DMAs are >=512B-per-partition runs (below that the cost doubles); hT streams
as [128, 1024] fp16 tiles so the first projection group completes ~6us in.
"""
import sys

if "/opt/trn_rl_repo" not in sys.path:
    sys.path.insert(0, "/opt/trn_rl_repo")

import numpy as np

import concourse.bass as bass
from concourse import bacc
import concourse.mybir as mybir
import concourse.tile as tile
from concourse.bass_utils import run_bass_kernel_spmd
from concourse.tile_rust import add_dep_helper

F32 = mybir.dt.float32
F16 = mybir.dt.float16  # fp16: same speed/DMA as bf16, 4x finer mantissa
AF = mybir.ActivationFunctionType
OP = mybir.AluOpType

B, T, D = 1, 2048, 2048
NH, NKV, HD = 16, 4, 128
EPS = 1e-6
SCALE = HD ** -0.5
NCORES = 8
P = 128
NJ = T // 512      # 4 t-chunks of 512
NT = T // P        # 16 s-tiles of 128
DT = D // P        # 16 contraction tiles
TSL = T // NCORES  # 256 output rows per core
EXP_BIAS = -4.0    # exp(st-4): keeps fp16 probs < 65504; cancels in ratio

_program_cache: dict = {}


def _tile_flags(seg_end: np.ndarray):
    """Per (s-tile i, t-chunk j): None if skipped, else (c0, c1, needs_c,
    needs_s). Valid query cols are [c0, c1): c0 from causality (queries >=
    tile's first key), c1 from segments (all keys' segments end by
    seg_end(last key))."""
    out = []
    for i in range(NT):
        smin, smax = P * i, P * i + P - 1
        se_lo, se_hi = int(seg_end[smin]), int(seg_end[smax])
        row = []
        for j in range(NJ):
            c0 = max(0, P * i - 512 * j)
            c1 = min(512, se_hi - 512 * j)
            if c1 <= c0:
                row.append(None)
            else:
                needs_c = (P * i - 512 * j) >= 0      # diagonal tile
                needs_s = (se_lo - 512 * j) < c1      # seg boundary inside
                row.append((c0, c1, needs_c, needs_s))
        out.append(tuple(row))
    return tuple(out)


def _build_program(key, use_collective=True):
    flags, unit_w = key
    nc = bacc.Bacc("TRN2", target_bir_lowering=False, debug=False,
                   num_devices=NCORES)

    hT_d = nc.dram_tensor("hT", [D, T], F16, kind="ExternalInput")
    # host-prepacked partition-major weights (see _host_prep)
    wqg_d = nc.dram_tensor("wqg", [P, DT, 512], F16, kind="ExternalInput")
    wkv_d = nc.dram_tensor("wkv", [P, DT, 256], F16, kind="ExternalInput")
    wo_d = nc.dram_tensor("wo", [P, NT, 2048], F16, kind="ExternalInput")
    tblq_d = nc.dram_tensor("tblq", [2, P, T], F16, kind="ExternalInput")
    if not unit_w:
        wqk_d = nc.dram_tensor("wqk", [P, 2], F32, kind="ExternalInput")
    iota_d = nc.dram_tensor("iota", [P, 512], F16, kind="ExternalInput")
    caus_d = nc.dram_tensor("caus", [P, 512], F16, kind="ExternalInput")
    segrel_d = nc.dram_tensor("segrel", [P, NT, NJ], F16, kind="ExternalInput")
    out_d = nc.dram_tensor("out", [TSL, D], F16, kind="ExternalOutput")

    hT_re = hT_d.rearrange("(dt p) t -> p dt t", p=P)

    with tile.TileContext(nc) as tc:
        with (
            tc.tile_pool(name="consts", bufs=1) as consts,
            tc.tile_pool(name="perm", bufs=1) as perm,
            tc.tile_pool(name="hw", bufs=32) as hw,
            tc.tile_pool(name="wop", bufs=8) as wop,
            tc.tile_pool(name="tmp", bufs=5) as tmp,
            tc.tile_pool(name="ptp", bufs=8) as ptp,
            tc.tile_pool(name="ps", bufs=1, space="PSUM") as psp,
            tc.tile_pool(name="dram", bufs=1, space="DRAM") as dram,
        ):
            # ---- constants ----
            wqg_sb = [consts.tile([P, 4, 512], F16, tag="wqg", bufs=4,
                                  name=f"wqg{g}") for g in range(4)]
            wkv_sb = [consts.tile([P, 8, 256], F16, tag="wkv", bufs=2,
                                  name=f"wkv{g}") for g in range(2)]

            def wq_ap(dt, col0):
                return wqg_sb[dt // 4][:, dt % 4, col0:col0 + 128]

            def wkv_ap(dt, col0):
                return wkv_sb[dt // 8][:, dt % 8, col0:col0 + 128]

            tb = {}
            for nm, idx in (("cq", 0), ("sq", 1)):
                tb[nm] = consts.tile([P, T], F16, tag=f"tb_{nm}", name=f"tb_{nm}")
            if not unit_w:
                wqk_sb = consts.tile([P, 2], F32)
            iota_sb = consts.tile([P, 512], F16)
            caus_sb = consts.tile([P, 512], F16)
            segrel_sb = consts.tile([P, NT, NJ], F16)
            ones_f32 = consts.tile([P, P], F32)
            ones_sb = consts.tile([P, P], F16)
            eps_sb = consts.tile([P, 1], F32)
            ebias_sb = consts.tile([P, 1], F32)

            # ---- persistent activations ----
            qTr = [perm.tile([P, T], F16, tag=f"qTr{h}", name=f"qTr{h}")
                   for h in range(2)]
            kTr = perm.tile([P, T], F16, tag="kTr")
            gT = [perm.tile([P, T], F16, tag=f"gT{h}", name=f"gT{h}")
                  for h in range(2)]
            v_sb = perm.tile([P, NT, P], F16, tag="v_sb")

            # split A2A by head: h0's collective runs while h1 computes
            a2a_in = [dram.tile([NCORES * P, TSL], F16, name=f"a2a_in{h}")
                      for h in range(2)]
            a2a_in8 = [a.rearrange("(s r) t -> s r t", r=P) for a in a2a_in]
            a2a_out = [dram.tile([NCORES * P, TSL], F16, name=f"a2a_out{h}")
                       for h in range(2)]

            # ======== DMA emission (SP queue order = priority order) ========
            nc.sync.dma_start(wkv_sb[0][:], wkv_d[:, 0:8, :])
            hTt = [[None] * DT for _ in range(2)]
            for half in range(2):
                for dt in range(DT):
                    t_ = hw.tile([P, 1024], F16, tag="hw", bufs=32,
                                 name=f"hT_{half}_{dt}")
                    nc.sync.dma_start(
                        t_[:], hT_re[:, dt, 1024 * half:1024 * half + 1024])
                    hTt[half][dt] = t_
                    if half == 0:
                        if dt == 1:
                            for nm, idx in (("cq", 0), ("sq", 1)):
                                nc.sync.dma_start(tb[nm][:], tblq_d[idx])
                        if dt % 4 == 3:
                            g = dt // 4
                            nc.sync.dma_start(wqg_sb[g][:],
                                              wqg_d[:, 4 * g:4 * g + 4, :])
                        if dt == 8:
                            nc.sync.dma_start(wkv_sb[1][:], wkv_d[:, 8:16, :])
                        if dt == 12:
                            nc.sync.dma_start(iota_sb[:], iota_d[:])
                            nc.sync.dma_start(caus_sb[:], caus_d[:])
                            nc.sync.dma_start(segrel_sb[:], segrel_d[:])
                            if not unit_w:
                                nc.sync.dma_start(wqk_sb[:], wqk_d[:])
            # o-proj weights, first 8 blocks prefetched (bufs=8)
            wo_sb = [None] * NT
            for ht in range(8):
                w_ = wop.tile([P, 2048], F16, tag="wop", bufs=8,
                              name=f"wo_{ht}")
                nc.sync.dma_start(w_[:], wo_d[:, ht, :])
                wo_sb[ht] = w_

            # ---- small on-chip constants ----
            nc.vector.memset(ones_f32[:], 1.0)
            nc.vector.tensor_copy(ones_sb[:], ones_f32[:])
            nc.vector.memset(eps_sb[:], EPS)
            nc.vector.memset(ebias_sb[:], EXP_BIAS)

            # ================= projections =================
            def emit_v(j):
                # v directly in [token, hd] layout: hT tile is the stationary
                # side, so no PE transposes (and no serial aux-bank chain)
                half = j // 2
                for kk in range(4):
                    tt = 4 * j + kk
                    csl = slice((j % 2) * 512 + 128 * kk,
                                (j % 2) * 512 + 128 * kk + 128)
                    vacc = psp.tile([P, 128], F32, tag="acc", bufs=4,
                                    name=f"vacc_{tt}")
                    for dt in range(DT):
                        nc.tensor.matmul(vacc[:], hTt[half][dt][:, csl],
                                         wkv_ap(dt, 128),
                                         start=(dt == 0), stop=(dt == DT - 1))
                    nc.vector.tensor_copy(v_sb[:, tt, :], vacc[:])

            def emit_proj(c, j, dep=None):
                """c: 0=q0 1=q1 2=k 4=g0 5=g1"""
                half, jj = j // 2, j % 2
                tsl = slice(512 * j, 512 * j + 512)
                hsl = slice(512 * jj, 512 * jj + 512)
                if c < 2:
                    w_ap = lambda dt: wq_ap(dt, 128 * c)
                elif c == 2:
                    w_ap = lambda dt: wkv_ap(dt, 0)
                else:
                    w_ap = lambda dt: wq_ap(dt, 256 + 128 * (c - 4))

                ptag, pbufs = (("mm", 3) if c in (0, 1, 4, 5) else ("acc", 4))
                mm_ps = psp.tile([P, 512], F32, tag=ptag, bufs=pbufs,
                                 name=f"proj_{j}_{c}")
                for dt in range(DT):
                    mm = nc.tensor.matmul(mm_ps[:], w_ap(dt),
                                          hTt[half][dt][:, hsl],
                                          start=(dt == 0), stop=(dt == DT - 1))
                    if dep is not None and dt == 0:
                        add_dep_helper(mm.ins, dep, reason="phase order")

                if c in (0, 1, 2):  # q0/q1/k: rms-norm + rope
                    dest = qTr[c][:, tsl] if c < 2 else kTr[:, tsl]
                    qpre = tmp.tile([P, 512], F32, tag="tmp")
                    nc.vector.tensor_copy(qpre[:], mm_ps[:])
                    q2 = tmp.tile([P, 512], F16, tag="tmp2", bufs=2)
                    # square on DVE, keeping the Act engine free for the
                    # attention exps it bottlenecks on
                    nc.vector.tensor_tensor(q2[:], qpre[:], qpre[:], OP.mult)
                    if not unit_w:
                        # norm weight applied after the rms statistic,
                        # before rope (rope commutes with rsqrt only)
                        qw = tmp.tile([P, 512], F32, tag="tmp")
                        nc.vector.tensor_scalar_mul(
                            qw[:], qpre[:],
                            wqk_sb[:, (0 if c < 2 else 1):
                                   (1 if c < 2 else 2)])
                        qpre = qw
                    ssq_ps = psp.tile([P, 512], F32, tag="aux", bufs=1)
                    nc.tensor.matmul(ssq_ps[:], ones_sb[:], q2[:],
                                     start=True, stop=True)
                    rsv = tmp.tile([P, 512], F32, tag="tmp")
                    nc.scalar.activation(rsv[:], ssq_ps[:], AF.Ln,
                                         scale=1.0 / HD, bias=eps_sb[:, 0:1])
                    nc.scalar.activation(rsv[:], rsv[:], AF.Exp, scale=-0.5)
                    tcos = tmp.tile([P, 512], F32, tag="tmp")
                    nc.vector.tensor_tensor(tcos[:], qpre[:], tb["cq"][:, tsl],
                                            OP.mult)
                    t2 = tmp.tile([P, 512], F32, tag="tmp")
                    # sin table halves are pre-swapped host-side so both
                    # inputs share a base partition; only out is shifted
                    nc.vector.tensor_tensor(t2[0:64, :], qpre[64:128, :],
                                            tb["sq"][64:128, tsl], OP.mult)
                    nc.vector.tensor_tensor(t2[64:128, :], qpre[0:64, :],
                                            tb["sq"][0:64, tsl], OP.mult)
                    nc.vector.tensor_tensor(t2[:], tcos[:], t2[:], OP.add)
                    nc.vector.tensor_tensor(dest, t2[:], rsv[:], OP.mult)
                else:  # gate: store ln(1+exp(-g))
                    eg = tmp.tile([P, 512], F32, tag="tmp")
                    nc.scalar.activation(eg[:], mm_ps[:], AF.Exp, scale=-1.0)
                    nc.scalar.activation(gT[c - 4][:, tsl], eg[:],
                                         AF.Ln, bias=1.0)

            # ================= attention =================
            # Two chunks emitted round-robin, with the ot/rs accumulation
            # matmuls trailing the st/exp/mask pipeline by ACC_LAG tiles: by
            # the time an accumulation reaches the PE sequencer its masked-pt
            # input is ready, so it flows through to the deep exec queue
            # instead of parking in the 4-slot wait queue and head-of-line
            # blocking the (ready) st matmuls behind it.
            ACC_LAG = 4
            acc_anchor = {}  # h -> last accumulation matmul instruction
            st_anchor = {}   # h -> last st matmul instruction

            def emit_gating(h, j, ot_ps, rs_ps):
                # sig(g)/rowsum = exp(-(ln(1+e^-g) + ln(rowsum)));
                # gT already holds ln(1+e^-g)
                tsl = slice(512 * j, 512 * j + 512)
                sg = tmp.tile([P, 512], F32, tag="tmpg", bufs=6,
                              name=f"sg_{h}_{j}")
                nc.scalar.activation(sg[:], rs_ps[:], AF.Ln)
                nc.vector.tensor_tensor(sg[:], sg[:], gT[h][:, tsl], OP.add)
                nc.scalar.activation(sg[:], sg[:], AF.Exp, scale=-1.0)
                atg = tmp.tile([P, 512], F16, tag="tmpg", bufs=6,
                               name=f"atg_{h}_{j}")
                nc.vector.tensor_tensor(atg[:], ot_ps[:], sg[:], OP.mult)
                # stage into a2a_in[h]: chunk j covers shards 2j and 2j+1
                for half in range(2):
                    nc.sync.dma_start(
                        a2a_in8[h][2 * j + half, :, :],
                        atg[:, 256 * half:256 * half + 256])

            def emit_attention_pair(h, jA, jB):
                state = {}
                for j in (jA, jB):
                    state[j] = dict(
                        valid=[(i,) + flags[i][j] for i in range(NT)
                               if flags[i][j] is not None],
                        ot=psp.tile([P, 512], F32, tag="acc", bufs=4,
                                    name=f"ot_{h}_{j}"),
                        rs=psp.tile([P, 512], F32, tag="acc", bufs=4,
                                    name=f"rs_{h}_{j}"),
                        maxc1=0, emitted=0)
                # merged round-robin order of (j, tile-idx)
                seq = []
                nA, nB = len(state[jA]["valid"]), len(state[jB]["valid"])
                for k in range(max(nA, nB)):
                    if k < nA:
                        seq.append((jA, k))
                    if k < nB:
                        seq.append((jB, k))
                pts = {}

                def emit_front(j, idx):
                    i, c0, c1, needs_c, needs_s = state[j]["valid"][idx]
                    tsl0 = 512 * j
                    st_ps = psp.tile([P, 512], F32, tag="mm", bufs=3,
                                     name=f"st_{h}_{j}_{i}")
                    st_anchor[h] = nc.tensor.matmul(
                        st_ps[:, c0:c1],
                        kTr[:, P * i:P * i + P],
                        qTr[h][:, tsl0 + c0:tsl0 + c1],
                        start=True, stop=True).ins
                    pt = ptp.tile([P, 512], F16, tag="pt",
                                  name=f"pt_{h}_{j}_{i}")
                    nc.scalar.activation(pt[:, c0:c1], st_ps[:, c0:c1],
                                         AF.Exp, bias=ebias_sb[:, 0:1])
                    if needs_c:
                        # diagonal tiles always have c0 == 128i - 512j, so
                        # the valid region relative to the slice start is the
                        # fixed staircase (col-offset >= partition): one fp16
                        # template multiply (2x DVE mode) replaces the Pool
                        # affine_select
                        nc.vector.tensor_tensor(pt[:, c0:c1], pt[:, c0:c1],
                                                caus_sb[:, 0:c1 - c0], OP.mult)
                    if needs_s:
                        nc.vector.scalar_tensor_tensor(
                            out=pt[:, c0:c1], in0=iota_sb[:, c0:c1],
                            scalar=segrel_sb[:, i, j:j + 1], in1=pt[:, c0:c1],
                            op0=OP.is_lt, op1=OP.mult)
                    pts[(j, idx)] = pt

                def emit_acc(j, idx):
                    s = state[j]
                    i, c0, c1, _, _ = s["valid"][idx]
                    pt = pts.pop((j, idx))
                    last = idx == len(s["valid"]) - 1
                    # coverage-split: first writer of each column range gets
                    # start=True (c0 is nondecreasing over valid tiles and
                    # every column's own diagonal tile is always valid, so
                    # ranges never leave gaps)
                    maxc1 = s["maxc1"]
                    if c1 <= maxc1:
                        segs = [(c0, c1, False)]
                    elif c0 < maxc1:
                        segs = [(c0, maxc1, False), (maxc1, c1, True)]
                    else:
                        segs = [(c0, c1, True)]
                    for (a, b, st_flag) in segs:
                        nc.tensor.matmul(s["ot"][:, a:b], v_sb[:, i, :],
                                         pt[:, a:b], start=st_flag,
                                         stop=last, skip_group_check=True)
                        acc_anchor[h] = nc.tensor.matmul(
                            s["rs"][:, a:b], ones_sb[:],
                            pt[:, a:b], start=st_flag,
                            stop=last, skip_group_check=True).ins
                    s["maxc1"] = max(maxc1, c1)
                    if last:
                        emit_gating(h, j, s["ot"], s["rs"])

                for k, (j, idx) in enumerate(seq):
                    emit_front(j, idx)
                    if k >= ACC_LAG:
                        emit_acc(*seq[k - ACC_LAG])
                for k in range(max(0, len(seq) - ACC_LAG), len(seq)):
                    emit_acc(*seq[k])

            # ======== phase A: k/v/q0/g0 for all T, then h0 attention ======
            for j in range(NJ):
                emit_proj(2, j)  # k
                emit_v(j)
                emit_proj(0, j)  # q0
                emit_proj(4, j)  # g0
            emit_attention_pair(0, 0, 3)
            emit_attention_pair(0, 1, 2)
            if use_collective:
                nc.gpsimd.collective_compute(
                    "AllToAll", OP.bypass,
                    replica_groups=[list(range(NCORES))],
                    ins=[a2a_in[0][:].opt()], outs=[a2a_out[0][:].opt()])
            else:
                nc.sync.dma_start(a2a_out[0][:], a2a_in[0][:])

            # ======== phase C: q1/g1, then h1 attention (over coll0) =======
            for j in range(NJ):
                for c in (1, 5):  # q1, g1
                    emit_proj(c, j)
            emit_attention_pair(1, 0, 3)
            emit_attention_pair(1, 1, 2)
            if use_collective:
                nc.gpsimd.collective_compute(
                    "AllToAll", OP.bypass,
                    replica_groups=[list(range(NCORES))],
                    ins=[a2a_in[1][:].opt()], outs=[a2a_out[1][:].opt()])
            else:
                nc.sync.dma_start(a2a_out[1][:], a2a_in[1][:])

            # ================= o-proj =================
            # ht order: all 8 h0 blocks (ready at coll0), then 8 h1 blocks;
            # wo is host-packed to match. ATall DMAs interleave with the
            # remaining wo loads so nothing dep-blocks the queue head.
            # one strided DMA per head gathers all 8 [128, 256] blocks
            # (vs 8 separate DMAs paying 625ns HWDGE generation each)
            a2a_out_r = [a.rearrange("(s r) t -> r s t", r=P) for a in a2a_out]
            ATg = []
            for h2 in range(2):
                at_t = perm.tile([P, 8, TSL], F16, tag="ATall", bufs=2,
                                 name=f"ATall{h2}")
                nc.sync.dma_start(at_t[:], a2a_out_r[h2][:])
                ATg.append(at_t)
                for ht in range(8 + 4 * h2, 12 + 4 * h2):
                    w_ = wop.tile([P, 2048], F16, tag="wop", bufs=8,
                                  name=f"wo_{ht}")
                    nc.sync.dma_start(w_[:], wo_d[:, ht, :])
                    wo_sb[ht] = w_

            # all 8 PSUM banks accumulate [m 0/1] x [Dc 0..3]
            ops_tags = ["mm", "mm", "mm", "aux", "acc", "acc", "acc", "acc"]
            ops_bufs = {"mm": 3, "aux": 1, "acc": 4}
            ops = []
            for m in range(2):
                for Dc in range(NJ):
                    tg = ops_tags[m * NJ + Dc]
                    ops.append(psp.tile([P, 512], F32, tag=tg,
                                        bufs=ops_bufs[tg], name=f"ops{m}_{Dc}"))
            o_sb = [tmp.tile([P, NJ, 512], F16, tag="osb", bufs=2,
                             name=f"osb_{m}") for m in range(2)]
            for ht in range(NT):
                at_t = ATg[ht // 8]
                w_full = wo_sb[ht]
                for Dc in range(NJ):
                    for m in range(2):
                        mm = nc.tensor.matmul(
                            ops[m * NJ + Dc][:],
                            at_t[:, ht % 8, 128 * m:128 * m + 128],
                            w_full[:, 512 * Dc:512 * Dc + 512],
                            start=(ht == 0), stop=(ht == NT - 1))
                        if ht == 0:
                            # keep o-proj out of the PE stream until h1's
                            # attention logits are all issued: the scheduler
                            # otherwise slots these (collective-gated) ahead
                            # of ready attention work and stalls the PE; the
                            # trailing accumulations interleave fine
                            add_dep_helper(mm.ins, st_anchor[1],
                                           reason="oproj after attn1")
                        if ht == NT - 1:
                            # assemble each finished accumulator into the
                            # per-m output tile, alternating Act/DVE so the
                            # copies run in parallel on both engines
                            if Dc % 2 == 0:
                                nc.vector.tensor_copy(o_sb[m][:, Dc, :],
                                                      ops[m * NJ + Dc][:])
                            else:
                                nc.scalar.activation(o_sb[m][:, Dc, :],
                                                     ops[m * NJ + Dc][:],
                                                     AF.Copy)
            for m in range(2):
                nc.sync.dma_start(out_d[128 * m:128 * m + 128, :], o_sb[m][:])

    nc.compile()
    _dedupe_act_table_loads(nc)
    return nc


def _dedupe_act_table_loads(nc):
    """Bacc assigns Exp->exp_and_others and Ln->natural_log, inserting a
    ~2.7us table load at every Exp<->Ln alternation. All activation funcs
    this kernel uses (Exp, Ln, Square) live in the natural_log_exp_and_others
    set, so keep one load of that set and drop the rest."""
    from concourse.hw_specs import get_activation_tables
    tabs = list(get_activation_tables(nc.m.arch).items())
    nl_exp = next(i for i, (nm, funcs) in enumerate(tabs)
                  if nm == "natural_log_exp_and_others")
    used = {ins.func for bb in nc.main_func.blocks for ins in bb.instructions
            if isinstance(ins, mybir.InstActivation)}
    assert used <= tabs[nl_exp][1], f"funcs {used} not all in natural_log_exp"
    first = True
    for bb in nc.main_func.blocks:
        keep = []
        for ins in bb.instructions:
            if isinstance(ins, mybir.InstLoadActFuncSet):
                assert ins.sync_info is None or (
                    not ins.sync_info.on_wait and not ins.sync_info.on_update)
                if first:
                    ins.act_func_set_id = nl_exp
                    keep.append(ins)
                    first = False
                continue
            keep.append(ins)
        bb.instructions[:] = keep


def _host_prep(hidden_BTD, cos_BTK, sin_BTK, segment_ids_BT, position_ids_BT,
               wq, wk, wv, wo, q_norm_w, k_norm_w):
    hidden = np.ascontiguousarray(np.asarray(hidden_BTD, dtype=np.float32)[0])
    cos = np.asarray(cos_BTK, dtype=np.float32)[0]
    sin = np.asarray(sin_BTK, dtype=np.float32)[0]
    seg = np.asarray(segment_ids_BT)[0]
    pos = np.asarray(position_ids_BT)[0]
    wq = np.asarray(wq, dtype=np.float32)
    wk = np.asarray(wk, dtype=np.float32)
    wv = np.asarray(wv, dtype=np.float32)
    wo = np.asarray(wo, dtype=np.float32)
    q_norm_w = np.asarray(q_norm_w, dtype=np.float32)
    k_norm_w = np.asarray(k_norm_w, dtype=np.float32)

    assert np.array_equal(pos, np.arange(T, dtype=pos.dtype)), \
        "kernel assumes position_ids == arange"
    assert np.all(np.diff(seg) >= 0), "kernel assumes sorted segment ids"

    hT = np.ascontiguousarray(hidden.T.astype(np.float16))
    sqrtS = np.float32(np.sqrt(SCALE))
    signv = np.where(np.arange(HD) < HD // 2, -1.0, 1.0).astype(np.float32)
    shuf = (np.arange(HD) + HD // 2) % HD

    cosw = (cos.T * sqrtS).astype(np.float32)
    sinw = (sin.T * signv[:, None] * sqrtS).astype(np.float32)
    sinswap = sinw[shuf]  # halves swapped: see rotate-half ops in _build_program
    tblq = np.ascontiguousarray(np.stack([cosw, sinswap]).astype(np.float16))
    unit_w = bool(np.all(q_norm_w == 1.0) and np.all(k_norm_w == 1.0))
    wqk = np.ascontiguousarray(np.stack([q_norm_w, k_norm_w], axis=1))

    # prepack wo into partition-major layout; block order matches the
    # o-proj ht-step order (all h0 head-blocks, then all h1)
    perm = [2 * i + h for h in range(2) for i in range(NCORES)]
    wo_p = wo.reshape(NT, P, 2048)[perm].transpose(1, 0, 2)
    wo_p = np.ascontiguousarray(wo_p.astype(np.float16))

    seg_end = np.searchsorted(seg, seg, side="right").astype(np.int64)
    iota = np.broadcast_to(np.arange(512, dtype=np.float16), (P, 512)).copy()
    # causal staircase: for a diagonal tile the valid region relative to the
    # slice start is always (col-offset >= partition)
    caus = (np.arange(512)[None, :] >= np.arange(P)[:, None]).astype(np.float16)
    segrel = np.zeros((P, NT, NJ), dtype=np.float16)
    for i in range(NT):
        for j in range(NJ):
            segrel[:, i, j] = seg_end[P * i:P * i + P] - 512.0 * j

    in_maps = []
    for c in range(NCORES):
        h0, h1 = 2 * c, 2 * c + 1
        g = c // 2
        wqg = np.concatenate([
            wq[:, h0 * 256: h0 * 256 + 128],
            wq[:, h1 * 256: h1 * 256 + 128],
            wq[:, h0 * 256 + 128: h0 * 256 + 256],
            wq[:, h1 * 256 + 128: h1 * 256 + 256],
        ], axis=1).astype(np.float16)
        wqg_p = np.ascontiguousarray(wqg.reshape(DT, P, 512).transpose(1, 0, 2))
        wkv = np.concatenate([
            wk[:, g * 128:(g + 1) * 128], wv[:, g * 128:(g + 1) * 128]],
            axis=1).astype(np.float16)
        wkv_p = np.ascontiguousarray(wkv.reshape(DT, P, 256).transpose(1, 0, 2))
        m = {
            "hT": hT, "wqg": wqg_p, "wkv": wkv_p, "wo": wo_p,
            "tblq": tblq, "iota": iota, "caus": caus, "segrel": segrel,
        }
        if not unit_w:
            m["wqk"] = wqk
        in_maps.append(m)
    return in_maps, seg_end, unit_w


def kernel(**inputs) -> np.ndarray:
    in_maps, seg_end, unit_w = _host_prep(**inputs)
    key = (_tile_flags(seg_end), unit_w)
    if key not in _program_cache:
        _program_cache[key] = _build_program(key)
    nc = _program_cache[key]
    res = run_bass_kernel_spmd(nc, in_maps, list(range(NCORES)))
    out = np.concatenate([res.results[c]["out"] for c in range(NCORES)], axis=0)
    return out[None].astype(np.float32)


# revision 49
# speedup vs baseline: 1.0088x; 1.0088x over previous
"""Trainium2 Bass kernel for nn_Attention_29738353557815.

8-way tensor-parallel over heads:
  - core c owns q-heads {2c, 2c+1} and kv-head c//2 (k/v proj duplicated per
    core pair); projections run weights-stationary off a host-pretransposed
    hidden^T in fp16, producing q/k in [head_dim, T] fp16 layout
  - rms-norm folded into ln/exp on ACT; rope tables (cos/sin * sqrt(scale))
    host-precomputed in [hd, T] layout; rotate-half via half-tile
    tensor_tensor ops against a half-swapped sin table
  - attention in S^T layout ([key, query] tiles), column-narrowed per tile to
    the valid [c0, c1) query range implied by causality and the (sorted)
    segment ids; exp has bias=-4 so fp16 probabilities cannot overflow
    (cancels between numerator and row-sum); softmax denominator via
    ones-matmul column sums; normalization and sigmoid gating fused into one
    multiply before the o-projection
  - phase order k/v/q0/g0 -> attn h0 -> AllToAll 0 -> q1/g1 -> attn h1 ->
    AllToAll 1 -> o-proj, so each 28us collective overlaps the other head's
    compute; o-proj consumes h0 blocks first so it starts right after coll0
  - o-proj writes psum accumulators straight to DRAM (2 KiB runs)

DMAs are >=512B-per-partition runs (below that the cost doubles); hT streams
as [128, 1024] fp16 tiles so the first projection group completes ~6us in.
"""
import sys

if "/opt/trn_rl_repo" not in sys.path:
    sys.path.insert(0, "/opt/trn_rl_repo")

import numpy as np

import concourse.bass as bass
from concourse import bacc
import concourse.mybir as mybir
import concourse.tile as tile
from concourse.bass_utils import run_bass_kernel_spmd
from concourse.tile_rust import add_dep_helper

F32 = mybir.dt.float32
F16 = mybir.dt.float16  # fp16: same speed/DMA as bf16, 4x finer mantissa
AF = mybir.ActivationFunctionType
OP = mybir.AluOpType

B, T, D = 1, 2048, 2048
NH, NKV, HD = 16, 4, 128
EPS = 1e-6
SCALE = HD ** -0.5
NCORES = 8
P = 128
NJ = T // 512      # 4 t-chunks of 512
NT = T // P        # 16 s-tiles of 128
DT = D // P        # 16 contraction tiles
TSL = T // NCORES  # 256 output rows per core
EXP_BIAS = -4.0    # exp(st-4): keeps fp16 probs < 65504; cancels in ratio

_program_cache: dict = {}


def _tile_flags(seg_end: np.ndarray):
    """Per (s-tile i, t-chunk j): None if skipped, else (c0, c1, needs_c,
    needs_s). Valid query cols are [c0, c1): c0 from causality (queries >=
    tile's first key), c1 from segments (all keys' segments end by
    seg_end(last key))."""
    out = []
    for i in range(NT):
        smin, smax = P * i, P * i + P - 1
        se_lo, se_hi = int(seg_end[smin]), int(seg_end[smax])
        row = []
        for j in range(NJ):
            c0 = max(0, P * i - 512 * j)
            c1 = min(512, se_hi - 512 * j)
            if c1 <= c0:
                row.append(None)
            else:
                needs_c = (P * i - 512 * j) >= 0      # diagonal tile
                needs_s = (se_lo - 512 * j) < c1      # seg boundary inside
                row.append((c0, c1, needs_c, needs_s))
        out.append(tuple(row))
    return tuple(out)


def _build_program(key, use_collective=True):
    flags, unit_w = key
    nc = bacc.Bacc("TRN2", target_bir_lowering=False, debug=False,
                   num_devices=NCORES)

    hT_d = nc.dram_tensor("hT", [D, T], F16, kind="ExternalInput")
    # host-prepacked partition-major weights (see _host_prep)
    wqg_d = nc.dram_tensor("wqg", [P, DT, 512], F16, kind="ExternalInput")
    wkv_d = nc.dram_tensor("wkv", [P, DT, 256], F16, kind="ExternalInput")
    wo_d = nc.dram_tensor("wo", [P, NT, 2048], F16, kind="ExternalInput")
    tblq_d = nc.dram_tensor("tblq", [2, P, T], F16, kind="ExternalInput")
    if not unit_w:
        wqk_d = nc.dram_tensor("wqk", [P, 2], F32, kind="ExternalInput")
    iota_d = nc.dram_tensor("iota", [P, 512], F16, kind="ExternalInput")
    caus_d = nc.dram_tensor("caus", [P, 512], F16, kind="ExternalInput")
    segrel_d = nc.dram_tensor("segrel", [P, NT, NJ], F16, kind="ExternalInput")
    out_d = nc.dram_tensor("out", [TSL, D], F16, kind="ExternalOutput")

    hT_re = hT_d.rearrange("(dt p) t -> p dt t", p=P)

    with tile.TileContext(nc) as tc:
        with (
            tc.tile_pool(name="consts", bufs=1) as consts,
            tc.tile_pool(name="perm", bufs=1) as perm,
            tc.tile_pool(name="hw", bufs=32) as hw,
            tc.tile_pool(name="wop", bufs=8) as wop,
            tc.tile_pool(name="tmp", bufs=5) as tmp,
            tc.tile_pool(name="ptp", bufs=8) as ptp,
            tc.tile_pool(name="ps", bufs=1, space="PSUM") as psp,
            tc.tile_pool(name="dram", bufs=1, space="DRAM") as dram,
        ):
            # ---- constants ----
            wqg_sb = [consts.tile([P, 4, 512], F16, tag="wqg", bufs=4,
                                  name=f"wqg{g}") for g in range(4)]
            wkv_sb = [consts.tile([P, 8, 256], F16, tag="wkv", bufs=2,
                                  name=f"wkv{g}") for g in range(2)]

            def wq_ap(dt, col0):
                return wqg_sb[dt // 4][:, dt % 4, col0:col0 + 128]

            def wkv_ap(dt, col0):
                return wkv_sb[dt // 8][:, dt % 8, col0:col0 + 128]

            tb = {}
            for nm, idx in (("cq", 0), ("sq", 1)):
                tb[nm] = consts.tile([P, T], F16, tag=f"tb_{nm}", name=f"tb_{nm}")
            if not unit_w:
                wqk_sb = consts.tile([P, 2], F32)
            iota_sb = consts.tile([P, 512], F16)
            caus_sb = consts.tile([P, 512], F16)
            segrel_sb = consts.tile([P, NT, NJ], F16)
            ones_f32 = consts.tile([P, P], F32)
            ones_sb = consts.tile([P, P], F16)
            eps_sb = consts.tile([P, 1], F32)
            ebias_sb = consts.tile([P, 1], F32)

            # ---- persistent activations ----
            qTr = [perm.tile([P, T], F16, tag=f"qTr{h}", name=f"qTr{h}")
                   for h in range(2)]
            kTr = perm.tile([P, T], F16, tag="kTr")
            gT = [perm.tile([P, T], F16, tag=f"gT{h}", name=f"gT{h}")
                  for h in range(2)]
            v_sb = perm.tile([P, NT, P], F16, tag="v_sb")

            # split A2A by head: h0's collective runs while h1 computes
            a2a_in = [dram.tile([NCORES * P, TSL], F16, name=f"a2a_in{h}")
                      for h in range(2)]
            a2a_in8 = [a.rearrange("(s r) t -> s r t", r=P) for a in a2a_in]
            a2a_out = [dram.tile([NCORES * P, TSL], F16, name=f"a2a_out{h}")
                       for h in range(2)]

            # ======== DMA emission (SP queue order = priority order) ========
            nc.sync.dma_start(wkv_sb[0][:], wkv_d[:, 0:8, :])
            hTt = [[None] * DT for _ in range(2)]
            for half in range(2):
                for dt in range(DT):
                    t_ = hw.tile([P, 1024], F16, tag="hw", bufs=32,
                                 name=f"hT_{half}_{dt}")
                    nc.sync.dma_start(
                        t_[:], hT_re[:, dt, 1024 * half:1024 * half + 1024])
                    hTt[half][dt] = t_
                    if half == 0:
                        if dt == 1:
                            for nm, idx in (("cq", 0), ("sq", 1)):
                                nc.sync.dma_start(tb[nm][:], tblq_d[idx])
                        if dt % 4 == 3:
                            g = dt // 4
                            nc.sync.dma_start(wqg_sb[g][:],
                                              wqg_d[:, 4 * g:4 * g + 4, :])
                        if dt == 8:
                            nc.sync.dma_start(wkv_sb[1][:], wkv_d[:, 8:16, :])
                        if dt == 12:
                            nc.sync.dma_start(iota_sb[:], iota_d[:])
                            nc.sync.dma_start(caus_sb[:], caus_d[:])
                            nc.sync.dma_start(segrel_sb[:], segrel_d[:])
                            if not unit_w:
                                nc.sync.dma_start(wqk_sb[:], wqk_d[:])
            # o-proj weights, first 8 blocks prefetched (bufs=8)
            wo_sb = [None] * NT
            for ht in range(8):
                w_ = wop.tile([P, 2048], F16, tag="wop", bufs=8,
                              name=f"wo_{ht}")
                nc.sync.dma_start(w_[:], wo_d[:, ht, :])
                wo_sb[ht] = w_

            # ---- small on-chip constants ----
            nc.vector.memset(ones_f32[:], 1.0)
            nc.vector.tensor_copy(ones_sb[:], ones_f32[:])
            nc.vector.memset(eps_sb[:], EPS)
            nc.vector.memset(ebias_sb[:], EXP_BIAS)

            # ================= projections =================
            def emit_v(j):
                # v directly in [token, hd] layout: hT tile is the stationary
                # side, so no PE transposes (and no serial aux-bank chain)
                half = j // 2
                for kk in range(4):
                    tt = 4 * j + kk
                    csl = slice((j % 2) * 512 + 128 * kk,
                                (j % 2) * 512 + 128 * kk + 128)
                    vacc = psp.tile([P, 128], F32, tag="acc", bufs=4,
                                    name=f"vacc_{tt}")
                    for dt in range(DT):
                        nc.tensor.matmul(vacc[:], hTt[half][dt][:, csl],
                                         wkv_ap(dt, 128),
                                         start=(dt == 0), stop=(dt == DT - 1))
                    nc.vector.tensor_copy(v_sb[:, tt, :], vacc[:])

            def emit_proj(c, j, dep=None):
                """c: 0=q0 1=q1 2=k 4=g0 5=g1"""
                half, jj = j // 2, j % 2
                tsl = slice(512 * j, 512 * j + 512)
                hsl = slice(512 * jj, 512 * jj + 512)
                if c < 2:
                    w_ap = lambda dt: wq_ap(dt, 128 * c)
                elif c == 2:
                    w_ap = lambda dt: wkv_ap(dt, 0)
                else:
                    w_ap = lambda dt: wq_ap(dt, 256 + 128 * (c - 4))

                ptag, pbufs = (("mm", 3) if c in (0, 1, 4, 5) else ("acc", 4))
                mm_ps = psp.tile([P, 512], F32, tag=ptag, bufs=pbufs,
                                 name=f"proj_{j}_{c}")
                for dt in range(DT):
                    mm = nc.tensor.matmul(mm_ps[:], w_ap(dt),
                                          hTt[half][dt][:, hsl],
                                          start=(dt == 0), stop=(dt == DT - 1))
                    if dep is not None and dt == 0:
                        add_dep_helper(mm.ins, dep, reason="phase order")

                if c in (0, 1, 2):  # q0/q1/k: rms-norm + rope
                    dest = qTr[c][:, tsl] if c < 2 else kTr[:, tsl]
                    qpre = tmp.tile([P, 512], F32, tag="tmp")
                    nc.vector.tensor_copy(qpre[:], mm_ps[:])
                    q2 = tmp.tile([P, 512], F16, tag="tmp2", bufs=2)
                    # square on DVE, keeping the Act engine free for the
                    # attention exps it bottlenecks on
                    nc.vector.tensor_tensor(q2[:], qpre[:], qpre[:], OP.mult)
                    if not unit_w:
                        # norm weight applied after the rms statistic,
                        # before rope (rope commutes with rsqrt only)
                        qw = tmp.tile([P, 512], F32, tag="tmp")
                        nc.vector.tensor_scalar_mul(
                            qw[:], qpre[:],
                            wqk_sb[:, (0 if c < 2 else 1):
                                   (1 if c < 2 else 2)])
                        qpre = qw
                    ssq_ps = psp.tile([P, 512], F32, tag="aux", bufs=1)
                    nc.tensor.matmul(ssq_ps[:], ones_sb[:], q2[:],
                                     start=True, stop=True)
                    rsv = tmp.tile([P, 512], F32, tag="tmp")
                    nc.scalar.activation(rsv[:], ssq_ps[:], AF.Ln,
                                         scale=1.0 / HD, bias=eps_sb[:, 0:1])
                    nc.scalar.activation(rsv[:], rsv[:], AF.Exp, scale=-0.5)
                    tcos = tmp.tile([P, 512], F32, tag="tmp")
                    nc.vector.tensor_tensor(tcos[:], qpre[:], tb["cq"][:, tsl],
                                            OP.mult)
                    t2 = tmp.tile([P, 512], F32, tag="tmp")
                    # sin table halves are pre-swapped host-side so both
                    # inputs share a base partition; only out is shifted
                    nc.vector.tensor_tensor(t2[0:64, :], qpre[64:128, :],
                                            tb["sq"][64:128, tsl], OP.mult)
                    nc.vector.tensor_tensor(t2[64:128, :], qpre[0:64, :],
                                            tb["sq"][0:64, tsl], OP.mult)
                    nc.vector.tensor_tensor(t2[:], tcos[:], t2[:], OP.add)
                    nc.vector.tensor_tensor(dest, t2[:], rsv[:], OP.mult)
                else:  # gate: store ln(1+exp(-g))
                    eg = tmp.tile([P, 512], F32, tag="tmp")
                    nc.scalar.activation(eg[:], mm_ps[:], AF.Exp, scale=-1.0)
                    nc.scalar.activation(gT[c - 4][:, tsl], eg[:],
                                         AF.Ln, bias=1.0)

            # ================= attention =================
            # Two chunks emitted round-robin, with the ot/rs accumulation
            # matmuls trailing the st/exp/mask pipeline by ACC_LAG tiles: by
            # the time an accumulation reaches the PE sequencer its masked-pt
            # input is ready, so it flows through to the deep exec queue
            # instead of parking in the 4-slot wait queue and head-of-line
            # blocking the (ready) st matmuls behind it.
            ACC_LAG = 4
            acc_anchor = {}  # h -> last accumulation matmul instruction
            st_anchor = {}   # h -> last st matmul instruction

            def emit_gating(h, j, ot_ps, rs_ps):
                # sig(g)/rowsum = exp(-(ln(1+e^-g) + ln(rowsum)));
                # gT already holds ln(1+e^-g)
                tsl = slice(512 * j, 512 * j + 512)
                sg = tmp.tile([P, 512], F32, tag="tmpg", bufs=6,
                              name=f"sg_{h}_{j}")
                nc.scalar.activation(sg[:], rs_ps[:], AF.Ln)
                nc.vector.tensor_tensor(sg[:], sg[:], gT[h][:, tsl], OP.add)
                nc.scalar.activation(sg[:], sg[:], AF.Exp, scale=-1.0)
                atg = tmp.tile([P, 512], F16, tag="tmpg", bufs=6,
                               name=f"atg_{h}_{j}")
                nc.vector.tensor_tensor(atg[:], ot_ps[:], sg[:], OP.mult)
                # stage into a2a_in[h]: chunk j covers shards 2j and 2j+1
                for half in range(2):
                    nc.sync.dma_start(
                        a2a_in8[h][2 * j + half, :, :],
                        atg[:, 256 * half:256 * half + 256])

            def emit_attention_pair(h, jA, jB):
                state = {}
                for j in (jA, jB):
                    state[j] = dict(
                        valid=[(i,) + flags[i][j] for i in range(NT)
                               if flags[i][j] is not None],
                        ot=psp.tile([P, 512], F32, tag="acc", bufs=4,
                                    name=f"ot_{h}_{j}"),
                        rs=psp.tile([P, 512], F32, tag="acc", bufs=4,
                                    name=f"rs_{h}_{j}"),
                        maxc1=0, emitted=0)
                # merged round-robin order of (j, tile-idx)
                seq = []
                nA, nB = len(state[jA]["valid"]), len(state[jB]["valid"])
                for k in range(max(nA, nB)):
                    if k < nA:
                        seq.append((jA, k))
                    if k < nB:
                        seq.append((jB, k))
                pts = {}

                def emit_front(j, idx):
                    i, c0, c1, needs_c, needs_s = state[j]["valid"][idx]
                    tsl0 = 512 * j
                    st_ps = psp.tile([P, 512], F32, tag="mm", bufs=3,
                                     name=f"st_{h}_{j}_{i}")
                    st_anchor[h] = nc.tensor.matmul(
                        st_ps[:, c0:c1],
                        kTr[:, P * i:P * i + P],
                        qTr[h][:, tsl0 + c0:tsl0 + c1],
                        start=True, stop=True).ins
                    pt = ptp.tile([P, 512], F16, tag="pt",
                                  name=f"pt_{h}_{j}_{i}")
                    nc.scalar.activation(pt[:, c0:c1], st_ps[:, c0:c1],
                                         AF.Exp, bias=ebias_sb[:, 0:1])
                    if needs_c:
                        # diagonal tiles always have c0 == 128i - 512j, so
                        # the valid region relative to the slice start is the
                        # fixed staircase (col-offset >= partition): one fp16
                        # template multiply (2x DVE mode) replaces the Pool
                        # affine_select
                        nc.vector.tensor_tensor(pt[:, c0:c1], pt[:, c0:c1],
                                                caus_sb[:, 0:c1 - c0], OP.mult)
                    if needs_s:
                        nc.vector.scalar_tensor_tensor(
                            out=pt[:, c0:c1], in0=iota_sb[:, c0:c1],
                            scalar=segrel_sb[:, i, j:j + 1], in1=pt[:, c0:c1],
                            op0=OP.is_lt, op1=OP.mult)
                    pts[(j, idx)] = pt

                def emit_acc(j, idx):
                    s = state[j]
                    i, c0, c1, _, _ = s["valid"][idx]
                    pt = pts.pop((j, idx))
                    last = idx == len(s["valid"]) - 1
                    # coverage-split: first writer of each column range gets
                    # start=True (c0 is nondecreasing over valid tiles and
                    # every column's own diagonal tile is always valid, so
                    # ranges never leave gaps)
                    maxc1 = s["maxc1"]
                    if c1 <= maxc1:
                        segs = [(c0, c1, False)]
                    elif c0 < maxc1:
                        segs = [(c0, maxc1, False), (maxc1, c1, True)]
                    else:
                        segs = [(c0, c1, True)]
                    for (a, b, st_flag) in segs:
                        nc.tensor.matmul(s["ot"][:, a:b], v_sb[:, i, :],
                                         pt[:, a:b], start=st_flag,
                                         stop=last, skip_group_check=True)
                        acc_anchor[h] = nc.tensor.matmul(
                            s["rs"][:, a:b], ones_sb[:],
                            pt[:, a:b], start=st_flag,
                            stop=last, skip_group_check=True).ins
                    s["maxc1"] = max(maxc1, c1)
                    if last:
                        emit_gating(h, j, s["ot"], s["rs"])

                for k, (j, idx) in enumerate(seq):
                    emit_front(j, idx)
                    if k >= ACC_LAG:
                        emit_acc(*seq[k - ACC_LAG])
                for k in range(max(0, len(seq) - ACC_LAG), len(seq)):
                    emit_acc(*seq[k])

            # ======== phase A: k/v/q0/g0 for all T, then h0 attention ======
            for j in range(NJ):
                emit_proj(2, j)  # k
                emit_v(j)
                emit_proj(0, j)  # q0
                emit_proj(4, j)  # g0
            emit_attention_pair(0, 0, 3)
            emit_attention_pair(0, 1, 2)
            if use_collective:
                nc.gpsimd.collective_compute(
                    "AllToAll", OP.bypass,
                    replica_groups=[list(range(NCORES))],
                    ins=[a2a_in[0][:].opt()], outs=[a2a_out[0][:].opt()])
            else:
                nc.sync.dma_start(a2a_out[0][:], a2a_in[0][:])

            # ======== phase C: q1/g1, then h1 attention (over coll0) =======
            for j in range(NJ):
                for c in (1, 5):  # q1, g1
                    emit_proj(c, j)
            emit_attention_pair(1, 0, 3)
            emit_attention_pair(1, 1, 2)
            if use_collective:
                nc.gpsimd.collective_compute(
                    "AllToAll", OP.bypass,
                    replica_groups=[list(range(NCORES))],
                    ins=[a2a_in[1][:].opt()], outs=[a2a_out[1][:].opt()])
            else:
                nc.sync.dma_start(a2a_out[1][:], a2a_in[1][:])

            # ================= o-proj =================
            # ht order: all 8 h0 blocks (ready at coll0), then 8 h1 blocks;
            # wo is host-packed to match. ATall DMAs interleave with the
            # remaining wo loads so nothing dep-blocks the queue head.
            # one strided DMA per head gathers all 8 [128, 256] blocks
            # (vs 8 separate DMAs paying 625ns HWDGE generation each)
            a2a_out_r = [a.rearrange("(s r) t -> r s t", r=P) for a in a2a_out]
            ATg = []
            for h2 in range(2):
                at_t = perm.tile([P, 8, TSL], F16, tag="ATall", bufs=2,
                                 name=f"ATall{h2}")
                # two 4-block gathers so the first o-proj steps start sooner
                nc.sync.dma_start(at_t[:, 0:4, :], a2a_out_r[h2][:, 0:4, :])
                nc.sync.dma_start(at_t[:, 4:8, :], a2a_out_r[h2][:, 4:8, :])
                ATg.append(at_t)
                for ht in range(8 + 4 * h2, 12 + 4 * h2):
                    w_ = wop.tile([P, 2048], F16, tag="wop", bufs=8,
                                  name=f"wo_{ht}")
                    nc.sync.dma_start(w_[:], wo_d[:, ht, :])
                    wo_sb[ht] = w_

            # all 8 PSUM banks accumulate [m 0/1] x [Dc 0..3]
            ops_tags = ["mm", "mm", "mm", "aux", "acc", "acc", "acc", "acc"]
            ops_bufs = {"mm": 3, "aux": 1, "acc": 4}
            ops = []
            for m in range(2):
                for Dc in range(NJ):
                    tg = ops_tags[m * NJ + Dc]
                    ops.append(psp.tile([P, 512], F32, tag=tg,
                                        bufs=ops_bufs[tg], name=f"ops{m}_{Dc}"))
            o_sb = [tmp.tile([P, NJ, 512], F16, tag="osb", bufs=2,
                             name=f"osb_{m}") for m in range(2)]
            for ht in range(NT - 1):
                at_t = ATg[ht // 8]
                w_full = wo_sb[ht]
                for Dc in range(NJ):
                    for m in range(2):
                        mm = nc.tensor.matmul(
                            ops[m * NJ + Dc][:],
                            at_t[:, ht % 8, 128 * m:128 * m + 128],
                            w_full[:, 512 * Dc:512 * Dc + 512],
                            start=(ht == 0), stop=False)
                        if ht == 0:
                            # keep o-proj out of the PE stream until h1's
                            # attention logits are all issued: the scheduler
                            # otherwise slots these (collective-gated) ahead
                            # of ready attention work and stalls the PE; the
                            # trailing accumulations interleave fine
                            add_dep_helper(mm.ins, st_anchor[1],
                                           reason="oproj after attn1")
            # final contraction step m-major: m=0's output DMA leaves while
            # m=1's accumulators finish; copies alternate Act/DVE
            for m in range(2):
                for Dc in range(NJ):
                    nc.tensor.matmul(
                        ops[m * NJ + Dc][:],
                        ATg[1][:, 7, 128 * m:128 * m + 128],
                        wo_sb[NT - 1][:, 512 * Dc:512 * Dc + 512],
                        start=False, stop=True)
                    if Dc % 2 == 0:
                        nc.vector.tensor_copy(o_sb[m][:, Dc, :],
                                              ops[m * NJ + Dc][:])
                    else:
                        nc.scalar.activation(o_sb[m][:, Dc, :],
                                             ops[m * NJ + Dc][:], AF.Copy)
                nc.sync.dma_start(out_d[128 * m:128 * m + 128, :], o_sb[m][:])

    nc.compile()
    _dedupe_act_table_loads(nc)
    return nc


def _dedupe_act_table_loads(nc):
    """Bacc assigns Exp->exp_and_others and Ln->natural_log, inserting a
    ~2.7us table load at every Exp<->Ln alternation. All activation funcs
    this kernel uses (Exp, Ln, Square) live in the natural_log_exp_and_others
    set, so keep one load of that set and drop the rest."""
    from concourse.hw_specs import get_activation_tables
    tabs = list(get_activation_tables(nc.m.arch).items())
    nl_exp = next(i for i, (nm, funcs) in enumerate(tabs)
                  if nm == "natural_log_exp_and_others")
    used = {ins.func for bb in nc.main_func.blocks for ins in bb.instructions
            if isinstance(ins, mybir.InstActivation)}
    assert used <= tabs[nl_exp][1], f"funcs {used} not all in natural_log_exp"
    first = True
    for bb in nc.main_func.blocks:
        keep = []
        for ins in bb.instructions:
            if isinstance(ins, mybir.InstLoadActFuncSet):
                assert ins.sync_info is None or (
                    not ins.sync_info.on_wait and not ins.sync_info.on_update)
                if first:
                    ins.act_func_set_id = nl_exp
                    keep.append(ins)
                    first = False
                continue
            keep.append(ins)
        bb.instructions[:] = keep


def _host_prep(hidden_BTD, cos_BTK, sin_BTK, segment_ids_BT, position_ids_BT,
               wq, wk, wv, wo, q_norm_w, k_norm_w):
    hidden = np.ascontiguousarray(np.asarray(hidden_BTD, dtype=np.float32)[0])
    cos = np.asarray(cos_BTK, dtype=np.float32)[0]
    sin = np.asarray(sin_BTK, dtype=np.float32)[0]
    seg = np.asarray(segment_ids_BT)[0]
    pos = np.asarray(position_ids_BT)[0]
    wq = np.asarray(wq, dtype=np.float32)
    wk = np.asarray(wk, dtype=np.float32)
    wv = np.asarray(wv, dtype=np.float32)
    wo = np.asarray(wo, dtype=np.float32)
    q_norm_w = np.asarray(q_norm_w, dtype=np.float32)
    k_norm_w = np.asarray(k_norm_w, dtype=np.float32)

    assert np.array_equal(pos, np.arange(T, dtype=pos.dtype)), \
        "kernel assumes position_ids == arange"
    assert np.all(np.diff(seg) >= 0), "kernel assumes sorted segment ids"

    hT = np.ascontiguousarray(hidden.T.astype(np.float16))
    sqrtS = np.float32(np.sqrt(SCALE))
    signv = np.where(np.arange(HD) < HD // 2, -1.0, 1.0).astype(np.float32)
    shuf = (np.arange(HD) + HD // 2) % HD

    cosw = (cos.T * sqrtS).astype(np.float32)
    sinw = (sin.T * signv[:, None] * sqrtS).astype(np.float32)
    sinswap = sinw[shuf]  # halves swapped: see rotate-half ops in _build_program
    tblq = np.ascontiguousarray(np.stack([cosw, sinswap]).astype(np.float16))
    unit_w = bool(np.all(q_norm_w == 1.0) and np.all(k_norm_w == 1.0))
    wqk = np.ascontiguousarray(np.stack([q_norm_w, k_norm_w], axis=1))

    # prepack wo into partition-major layout; block order matches the
    # o-proj ht-step order (all h0 head-blocks, then all h1)
    perm = [2 * i + h for h in range(2) for i in range(NCORES)]
    wo_p = wo.reshape(NT, P, 2048)[perm].transpose(1, 0, 2)
    wo_p = np.ascontiguousarray(wo_p.astype(np.float16))

    seg_end = np.searchsorted(seg, seg, side="right").astype(np.int64)
    iota = np.broadcast_to(np.arange(512, dtype=np.float16), (P, 512)).copy()
    # causal staircase: for a diagonal tile the valid region relative to the
    # slice start is always (col-offset >= partition)
    caus = (np.arange(512)[None, :] >= np.arange(P)[:, None]).astype(np.float16)
    segrel = np.zeros((P, NT, NJ), dtype=np.float16)
    for i in range(NT):
        for j in range(NJ):
            segrel[:, i, j] = seg_end[P * i:P * i + P] - 512.0 * j

    in_maps = []
    for c in range(NCORES):
        h0, h1 = 2 * c, 2 * c + 1
        g = c // 2
        wqg = np.concatenate([
            wq[:, h0 * 256: h0 * 256 + 128],
            wq[:, h1 * 256: h1 * 256 + 128],
            wq[:, h0 * 256 + 128: h0 * 256 + 256],
            wq[:, h1 * 256 + 128: h1 * 256 + 256],
        ], axis=1).astype(np.float16)
        wqg_p = np.ascontiguousarray(wqg.reshape(DT, P, 512).transpose(1, 0, 2))
        wkv = np.concatenate([
            wk[:, g * 128:(g + 1) * 128], wv[:, g * 128:(g + 1) * 128]],
            axis=1).astype(np.float16)
        wkv_p = np.ascontiguousarray(wkv.reshape(DT, P, 256).transpose(1, 0, 2))
        m = {
            "hT": hT, "wqg": wqg_p, "wkv": wkv_p, "wo": wo_p,
            "tblq": tblq, "iota": iota, "caus": caus, "segrel": segrel,
        }
        if not unit_w:
            m["wqk"] = wqk
        in_maps.append(m)
    return in_maps, seg_end, unit_w


def kernel(**inputs) -> np.ndarray:
    in_maps, seg_end, unit_w = _host_prep(**inputs)
    key = (_tile_flags(seg_end), unit_w)
    if key not in _program_cache:
        _program_cache[key] = _build_program(key)
    nc = _program_cache[key]
    res = run_bass_kernel_spmd(nc, in_maps, list(range(NCORES)))
    out = np.concatenate([res.results[c]["out"] for c in range(NCORES)], axis=0)
    return out[None].astype(np.float32)


# revision 51
# speedup vs baseline: 1.0120x; 1.0032x over previous
"""Trainium2 Bass kernel for nn_Attention_29738353557815.

8-way tensor-parallel over heads:
  - core c owns q-heads {2c, 2c+1} and kv-head c//2 (k/v proj duplicated per
    core pair); projections run weights-stationary off a host-pretransposed
    hidden^T in fp16, producing q/k in [head_dim, T] fp16 layout
  - rms-norm folded into ln/exp on ACT; rope tables (cos/sin * sqrt(scale))
    host-precomputed in [hd, T] layout; rotate-half via half-tile
    tensor_tensor ops against a half-swapped sin table
  - attention in S^T layout ([key, query] tiles), column-narrowed per tile to
    the valid [c0, c1) query range implied by causality and the (sorted)
    segment ids; exp has bias=-4 so fp16 probabilities cannot overflow
    (cancels between numerator and row-sum); softmax denominator via
    ones-matmul column sums; normalization and sigmoid gating fused into one
    multiply before the o-projection
  - phase order k/v/q0/g0 -> attn h0 -> AllToAll 0 -> q1/g1 -> attn h1 ->
    AllToAll 1 -> o-proj, so each 28us collective overlaps the other head's
    compute; o-proj consumes h0 blocks first so it starts right after coll0
  - o-proj writes psum accumulators straight to DRAM (2 KiB runs)

DMAs are >=512B-per-partition runs (below that the cost doubles); hT streams
as [128, 1024] fp16 tiles so the first projection group completes ~6us in.
"""
import sys

if "/opt/trn_rl_repo" not in sys.path:
    sys.path.insert(0, "/opt/trn_rl_repo")

import numpy as np

import concourse.bass as bass
from concourse import bacc
import concourse.mybir as mybir
import concourse.tile as tile
from concourse.bass_utils import run_bass_kernel_spmd
from concourse.tile_rust import add_dep_helper

F32 = mybir.dt.float32
F16 = mybir.dt.float16  # fp16: same speed/DMA as bf16, 4x finer mantissa
AF = mybir.ActivationFunctionType
OP = mybir.AluOpType

B, T, D = 1, 2048, 2048
NH, NKV, HD = 16, 4, 128
EPS = 1e-6
SCALE = HD ** -0.5
NCORES = 8
P = 128
NJ = T // 512      # 4 t-chunks of 512
NT = T // P        # 16 s-tiles of 128
DT = D // P        # 16 contraction tiles
TSL = T // NCORES  # 256 output rows per core
EXP_BIAS = -4.0    # exp(st-4): keeps fp16 probs < 65504; cancels in ratio

_program_cache: dict = {}


def _tile_flags(seg_end: np.ndarray):
    """Per (s-tile i, t-chunk j): None if skipped, else (c0, c1, needs_c,
    needs_s). Valid query cols are [c0, c1): c0 from causality (queries >=
    tile's first key), c1 from segments (all keys' segments end by
    seg_end(last key))."""
    out = []
    for i in range(NT):
        smin, smax = P * i, P * i + P - 1
        se_lo, se_hi = int(seg_end[smin]), int(seg_end[smax])
        row = []
        for j in range(NJ):
            c0 = max(0, P * i - 512 * j)
            c1 = min(512, se_hi - 512 * j)
            if c1 <= c0:
                row.append(None)
            else:
                needs_c = (P * i - 512 * j) >= 0      # diagonal tile
                needs_s = (se_lo - 512 * j) < c1      # seg boundary inside
                row.append((c0, c1, needs_c, needs_s))
        out.append(tuple(row))
    return tuple(out)


def _build_program(key, use_collective=True):
    flags, unit_w = key
    nc = bacc.Bacc("TRN2", target_bir_lowering=False, debug=False,
                   num_devices=NCORES)

    hT_d = nc.dram_tensor("hT", [D, T], F16, kind="ExternalInput")
    # host-prepacked partition-major weights (see _host_prep)
    wqg_d = nc.dram_tensor("wqg", [P, DT, 512], F16, kind="ExternalInput")
    wkv_d = nc.dram_tensor("wkv", [P, DT, 256], F16, kind="ExternalInput")
    wo_d = nc.dram_tensor("wo", [P, NT, 2048], F16, kind="ExternalInput")
    tblq_d = nc.dram_tensor("tblq", [2, P, T], F16, kind="ExternalInput")
    if not unit_w:
        wqk_d = nc.dram_tensor("wqk", [P, 2], F32, kind="ExternalInput")
    iota_d = nc.dram_tensor("iota", [P, 512], F16, kind="ExternalInput")
    caus_d = nc.dram_tensor("caus", [P, 512], F16, kind="ExternalInput")
    segrel_d = nc.dram_tensor("segrel", [P, NT, NJ], F16, kind="ExternalInput")
    out_d = nc.dram_tensor("out", [TSL, D], F16, kind="ExternalOutput")

    hT_re = hT_d.rearrange("(dt p) t -> p dt t", p=P)

    with tile.TileContext(nc) as tc:
        with (
            tc.tile_pool(name="consts", bufs=1) as consts,
            tc.tile_pool(name="perm", bufs=1) as perm,
            tc.tile_pool(name="hw", bufs=32) as hw,
            tc.tile_pool(name="wop", bufs=8) as wop,
            tc.tile_pool(name="tmp", bufs=5) as tmp,
            tc.tile_pool(name="ptp", bufs=8) as ptp,
            tc.tile_pool(name="ps", bufs=1, space="PSUM") as psp,
            tc.tile_pool(name="dram", bufs=1, space="DRAM") as dram,
        ):
            # ---- constants ----
            wqg_sb = [consts.tile([P, 4, 512], F16, tag="wqg", bufs=4,
                                  name=f"wqg{g}") for g in range(4)]
            wkv_sb = [consts.tile([P, 8, 256], F16, tag="wkv", bufs=2,
                                  name=f"wkv{g}") for g in range(2)]

            def wq_ap(dt, col0):
                return wqg_sb[dt // 4][:, dt % 4, col0:col0 + 128]

            def wkv_ap(dt, col0):
                return wkv_sb[dt // 8][:, dt % 8, col0:col0 + 128]

            tb = {}
            for nm, idx in (("cq", 0), ("sq", 1)):
                tb[nm] = consts.tile([P, T], F16, tag=f"tb_{nm}", name=f"tb_{nm}")
            if not unit_w:
                wqk_sb = consts.tile([P, 2], F32)
            iota_sb = consts.tile([P, 512], F16)
            caus_sb = consts.tile([P, 512], F16)
            segrel_sb = consts.tile([P, NT, NJ], F16)
            ones_f32 = consts.tile([P, P], F32)
            ones_sb = consts.tile([P, P], F16)
            eps_sb = consts.tile([P, 1], F32)
            ebias_sb = consts.tile([P, 1], F32)

            # ---- persistent activations ----
            qTr = [perm.tile([P, T], F16, tag=f"qTr{h}", name=f"qTr{h}")
                   for h in range(2)]
            kTr = perm.tile([P, T], F16, tag="kTr")
            gT = [perm.tile([P, T], F16, tag=f"gT{h}", name=f"gT{h}")
                  for h in range(2)]
            v_sb = perm.tile([P, NT, P], F16, tag="v_sb")

            # split A2A by head: h0's collective runs while h1 computes
            a2a_in = [dram.tile([NCORES * P, TSL], F16, name=f"a2a_in{h}")
                      for h in range(2)]
            a2a_in8 = [a.rearrange("(s r) t -> s r t", r=P) for a in a2a_in]
            a2a_out = [dram.tile([NCORES * P, TSL], F16, name=f"a2a_out{h}")
                       for h in range(2)]

            # ======== DMA emission (SP queue order = priority order) ========
            nc.sync.dma_start(wkv_sb[0][:], wkv_d[:, 0:8, :])
            hTt = [[None] * DT for _ in range(2)]
            for half in range(2):
                for dt in range(DT):
                    t_ = hw.tile([P, 1024], F16, tag="hw", bufs=32,
                                 name=f"hT_{half}_{dt}")
                    nc.sync.dma_start(
                        t_[:], hT_re[:, dt, 1024 * half:1024 * half + 1024])
                    hTt[half][dt] = t_
                    if half == 0:
                        if dt == 1:
                            for nm, idx in (("cq", 0), ("sq", 1)):
                                nc.sync.dma_start(tb[nm][:], tblq_d[idx])
                        if dt % 4 == 3:
                            g = dt // 4
                            nc.sync.dma_start(wqg_sb[g][:],
                                              wqg_d[:, 4 * g:4 * g + 4, :])
                        if dt == 8:
                            nc.sync.dma_start(wkv_sb[1][:], wkv_d[:, 8:16, :])
                        if dt == 12:
                            nc.sync.dma_start(iota_sb[:], iota_d[:])
                            nc.sync.dma_start(caus_sb[:], caus_d[:])
                            nc.sync.dma_start(segrel_sb[:], segrel_d[:])
                            if not unit_w:
                                nc.sync.dma_start(wqk_sb[:], wqk_d[:])
            # o-proj weights, first 8 blocks prefetched (bufs=8)
            wo_sb = [None] * NT
            for ht in range(8):
                w_ = wop.tile([P, 2048], F16, tag="wop", bufs=8,
                              name=f"wo_{ht}")
                nc.sync.dma_start(w_[:], wo_d[:, ht, :])
                wo_sb[ht] = w_

            # ---- small on-chip constants ----
            nc.vector.memset(ones_f32[:], 1.0)
            nc.vector.tensor_copy(ones_sb[:], ones_f32[:])
            nc.vector.memset(eps_sb[:], EPS)
            nc.vector.memset(ebias_sb[:], EXP_BIAS)

            # ================= projections =================
            def emit_v(j):
                # v directly in [token, hd] layout: hT tile is the stationary
                # side, so no PE transposes (and no serial aux-bank chain)
                half = j // 2
                for kk in range(4):
                    tt = 4 * j + kk
                    csl = slice((j % 2) * 512 + 128 * kk,
                                (j % 2) * 512 + 128 * kk + 128)
                    vacc = psp.tile([P, 128], F32, tag="acc", bufs=4,
                                    name=f"vacc_{tt}")
                    for dt in range(DT):
                        nc.tensor.matmul(vacc[:], hTt[half][dt][:, csl],
                                         wkv_ap(dt, 128),
                                         start=(dt == 0), stop=(dt == DT - 1))
                    nc.vector.tensor_copy(v_sb[:, tt, :], vacc[:])

            def emit_proj(c, j, dep=None):
                """c: 0=q0 1=q1 2=k 4=g0 5=g1"""
                half, jj = j // 2, j % 2
                tsl = slice(512 * j, 512 * j + 512)
                hsl = slice(512 * jj, 512 * jj + 512)
                if c < 2:
                    w_ap = lambda dt: wq_ap(dt, 128 * c)
                elif c == 2:
                    w_ap = lambda dt: wkv_ap(dt, 0)
                else:
                    w_ap = lambda dt: wq_ap(dt, 256 + 128 * (c - 4))

                ptag, pbufs = (("mm", 3) if c in (0, 1, 4, 5) else ("acc", 4))
                mm_ps = psp.tile([P, 512], F32, tag=ptag, bufs=pbufs,
                                 name=f"proj_{j}_{c}")
                for dt in range(DT):
                    mm = nc.tensor.matmul(mm_ps[:], w_ap(dt),
                                          hTt[half][dt][:, hsl],
                                          start=(dt == 0), stop=(dt == DT - 1))
                    if dep is not None and dt == 0:
                        add_dep_helper(mm.ins, dep, reason="phase order")

                if c in (0, 1, 2):  # q0/q1/k: rms-norm + rope
                    dest = qTr[c][:, tsl] if c < 2 else kTr[:, tsl]
                    qpre = tmp.tile([P, 512], F32, tag="tmp")
                    nc.vector.tensor_copy(qpre[:], mm_ps[:])
                    q2 = tmp.tile([P, 512], F16, tag="tmp2", bufs=2)
                    # square on DVE, keeping the Act engine free for the
                    # attention exps it bottlenecks on
                    nc.vector.tensor_tensor(q2[:], qpre[:], qpre[:], OP.mult)
                    if not unit_w:
                        # norm weight applied after the rms statistic,
                        # before rope (rope commutes with rsqrt only)
                        qw = tmp.tile([P, 512], F32, tag="tmp")
                        nc.vector.tensor_scalar_mul(
                            qw[:], qpre[:],
                            wqk_sb[:, (0 if c < 2 else 1):
                                   (1 if c < 2 else 2)])
                        qpre = qw
                    ssq_ps = psp.tile([P, 512], F32, tag="aux", bufs=1)
                    nc.tensor.matmul(ssq_ps[:], ones_sb[:], q2[:],
                                     start=True, stop=True)
                    rsv = tmp.tile([P, 512], F32, tag="tmp")
                    nc.scalar.activation(rsv[:], ssq_ps[:], AF.Ln,
                                         scale=1.0 / HD, bias=eps_sb[:, 0:1])
                    nc.scalar.activation(rsv[:], rsv[:], AF.Exp, scale=-0.5)
                    tcos = tmp.tile([P, 512], F32, tag="tmp")
                    nc.vector.tensor_tensor(tcos[:], qpre[:], tb["cq"][:, tsl],
                                            OP.mult)
                    t2 = tmp.tile([P, 512], F32, tag="tmp")
                    # sin table halves are pre-swapped host-side so both
                    # inputs share a base partition; only out is shifted
                    nc.vector.tensor_tensor(t2[0:64, :], qpre[64:128, :],
                                            tb["sq"][64:128, tsl], OP.mult)
                    nc.vector.tensor_tensor(t2[64:128, :], qpre[0:64, :],
                                            tb["sq"][0:64, tsl], OP.mult)
                    nc.vector.tensor_tensor(t2[:], tcos[:], t2[:], OP.add)
                    nc.vector.tensor_tensor(dest, t2[:], rsv[:], OP.mult)
                else:  # gate: store ln(1+exp(-g))
                    eg = tmp.tile([P, 512], F32, tag="tmp")
                    nc.scalar.activation(eg[:], mm_ps[:], AF.Exp, scale=-1.0)
                    nc.scalar.activation(gT[c - 4][:, tsl], eg[:],
                                         AF.Ln, bias=1.0)

            # ================= attention =================
            # Two chunks emitted round-robin, with the ot/rs accumulation
            # matmuls trailing the st/exp/mask pipeline by ACC_LAG tiles: by
            # the time an accumulation reaches the PE sequencer its masked-pt
            # input is ready, so it flows through to the deep exec queue
            # instead of parking in the 4-slot wait queue and head-of-line
            # blocking the (ready) st matmuls behind it.
            ACC_LAG = 4
            acc_anchor = {}  # h -> last accumulation matmul instruction
            st_anchor = {}   # h -> last st matmul instruction

            def emit_gating(h, j, ot_ps, rs_ps):
                # sig(g)/rowsum = exp(-(ln(1+e^-g) + ln(rowsum)));
                # gT already holds ln(1+e^-g)
                tsl = slice(512 * j, 512 * j + 512)
                sg = tmp.tile([P, 512], F32, tag="tmpg", bufs=6,
                              name=f"sg_{h}_{j}")
                nc.scalar.activation(sg[:], rs_ps[:], AF.Ln)
                nc.vector.tensor_tensor(sg[:], sg[:], gT[h][:, tsl], OP.add)
                nc.scalar.activation(sg[:], sg[:], AF.Exp, scale=-1.0)
                atg = tmp.tile([P, 512], F16, tag="tmpg", bufs=6,
                               name=f"atg_{h}_{j}")
                nc.vector.tensor_tensor(atg[:], ot_ps[:], sg[:], OP.mult)
                # stage into a2a_in[h]: chunk j covers shards 2j and 2j+1
                for half in range(2):
                    nc.sync.dma_start(
                        a2a_in8[h][2 * j + half, :, :],
                        atg[:, 256 * half:256 * half + 256])

            def emit_attention_pair(h, jA, jB):
                state = {}
                for j in (jA, jB):
                    state[j] = dict(
                        valid=[(i,) + flags[i][j] for i in range(NT)
                               if flags[i][j] is not None],
                        ot=psp.tile([P, 512], F32, tag="acc", bufs=4,
                                    name=f"ot_{h}_{j}"),
                        rs=psp.tile([P, 512], F32, tag="acc", bufs=4,
                                    name=f"rs_{h}_{j}"),
                        maxc1=0, emitted=0)
                # merged round-robin order of (j, tile-idx)
                seq = []
                nA, nB = len(state[jA]["valid"]), len(state[jB]["valid"])
                for k in range(max(nA, nB)):
                    if k < nA:
                        seq.append((jA, k))
                    if k < nB:
                        seq.append((jB, k))
                pts = {}

                def width(j, idx):
                    _, c0, c1, _, _2 = state[j]["valid"][idx]
                    return c1 - c0

                # group the merged sequence into emission units: narrow tiles
                # are packed TIGHTLY side by side in one PSUM bank and share
                # ONE exp over the contiguous written span, cutting the Act
                # fixed cost on the chain
                units = []
                k = 0
                while k < len(seq):
                    unit, span = [seq[k]], width(*seq[k])
                    k += 1
                    while (k < len(seq) and len(unit) < 3
                           and span + width(*seq[k]) <= 512):
                        unit.append(seq[k])
                        span += width(*seq[k])
                        k += 1
                    units.append(unit)

                def emit_unit(unit):
                    members = []
                    off = 0
                    for (j, idx) in unit:
                        i, c0, c1, needs_c, needs_s = state[j]["valid"][idx]
                        members.append((off, j, idx, i, c0, c1,
                                        needs_c, needs_s))
                        off += c1 - c0
                    span = off
                    st_ps = psp.tile([P, 512], F32, tag="mm", bufs=3,
                                     name=f"stu_{h}_{members[0][1]}_{members[0][3]}")
                    for off, j, idx, i, c0, c1, _, _2 in members:
                        st_anchor[h] = nc.tensor.matmul(
                            st_ps[:, off:off + c1 - c0],
                            kTr[:, P * i:P * i + P],
                            qTr[h][:, 512 * j + c0:512 * j + c1],
                            start=True, stop=True,
                            skip_group_check=True).ins
                    pt = ptp.tile([P, 512], F16, tag="pt",
                                  name=f"ptu_{h}_{members[0][1]}_{members[0][3]}")
                    nc.scalar.activation(pt[:, 0:span], st_ps[:, 0:span],
                                         AF.Exp, bias=ebias_sb[:, 0:1])
                    for off, j, idx, i, c0, c1, needs_c, needs_s in members:
                        w = c1 - c0
                        if needs_c:
                            # diagonal tiles have c0 == 128i - 512j, so the
                            # valid region relative to the slice start is the
                            # fixed staircase (col-offset >= partition): one
                            # fp16 template multiply (2x DVE mode) replaces
                            # the Pool affine_select
                            nc.vector.tensor_tensor(pt[:, off:off + w],
                                                    pt[:, off:off + w],
                                                    caus_sb[:, 0:w], OP.mult)
                        if needs_s:
                            nc.vector.scalar_tensor_tensor(
                                out=pt[:, off:off + w],
                                in0=iota_sb[:, c0:c1],
                                scalar=segrel_sb[:, i, j:j + 1],
                                in1=pt[:, off:off + w],
                                op0=OP.is_lt, op1=OP.mult)
                        pts[(j, idx)] = (pt, off)

                def emit_acc(j, idx):
                    s = state[j]
                    i, c0, c1, _, _2 = s["valid"][idx]
                    pt, off = pts.pop((j, idx))
                    last = idx == len(s["valid"]) - 1
                    # coverage-split: first writer of each column range gets
                    # start=True (c0 is nondecreasing over valid tiles and
                    # every column's own diagonal tile is always valid, so
                    # ranges never leave gaps)
                    maxc1 = s["maxc1"]
                    if c1 <= maxc1:
                        segs = [(c0, c1, False)]
                    elif c0 < maxc1:
                        segs = [(c0, maxc1, False), (maxc1, c1, True)]
                    else:
                        segs = [(c0, c1, True)]
                    for (a, b, st_flag) in segs:
                        pa, pb = off + a - c0, off + b - c0
                        nc.tensor.matmul(s["ot"][:, a:b], v_sb[:, i, :],
                                         pt[:, pa:pb], start=st_flag,
                                         stop=last, skip_group_check=True)
                        acc_anchor[h] = nc.tensor.matmul(
                            s["rs"][:, a:b], ones_sb[:],
                            pt[:, pa:pb], start=st_flag,
                            stop=last, skip_group_check=True).ins
                    s["maxc1"] = max(maxc1, c1)
                    if last:
                        emit_gating(h, j, s["ot"], s["rs"])

                done = []
                for u, unit in enumerate(units):
                    emit_unit(unit)
                    done.extend(unit)
                    while len(done) > ACC_LAG:
                        emit_acc(*done.pop(0))
                for ji in done:
                    emit_acc(*ji)

            # ======== phase A: k/v/q0/g0 for all T, then h0 attention ======
            for j in range(NJ):
                emit_proj(2, j)  # k
                emit_v(j)
                emit_proj(0, j)  # q0
                emit_proj(4, j)  # g0
            emit_attention_pair(0, 0, 3)
            emit_attention_pair(0, 1, 2)
            if use_collective:
                nc.gpsimd.collective_compute(
                    "AllToAll", OP.bypass,
                    replica_groups=[list(range(NCORES))],
                    ins=[a2a_in[0][:].opt()], outs=[a2a_out[0][:].opt()])
            else:
                nc.sync.dma_start(a2a_out[0][:], a2a_in[0][:])

            # ======== phase C: q1/g1, then h1 attention (over coll0) =======
            for j in range(NJ):
                for c in (1, 5):  # q1, g1
                    emit_proj(c, j)
            emit_attention_pair(1, 0, 3)
            emit_attention_pair(1, 1, 2)
            if use_collective:
                nc.gpsimd.collective_compute(
                    "AllToAll", OP.bypass,
                    replica_groups=[list(range(NCORES))],
                    ins=[a2a_in[1][:].opt()], outs=[a2a_out[1][:].opt()])
            else:
                nc.sync.dma_start(a2a_out[1][:], a2a_in[1][:])

            # ================= o-proj =================
            # ht order: all 8 h0 blocks (ready at coll0), then 8 h1 blocks;
            # wo is host-packed to match. ATall DMAs interleave with the
            # remaining wo loads so nothing dep-blocks the queue head.
            # one strided DMA per head gathers all 8 [128, 256] blocks
            # (vs 8 separate DMAs paying 625ns HWDGE generation each)
            a2a_out_r = [a.rearrange("(s r) t -> r s t", r=P) for a in a2a_out]
            ATg = []
            for h2 in range(2):
                at_t = perm.tile([P, 8, TSL], F16, tag="ATall", bufs=2,
                                 name=f"ATall{h2}")
                # two 4-block gathers so the first o-proj steps start sooner
                nc.sync.dma_start(at_t[:, 0:4, :], a2a_out_r[h2][:, 0:4, :])
                nc.sync.dma_start(at_t[:, 4:8, :], a2a_out_r[h2][:, 4:8, :])
                ATg.append(at_t)
                for ht in range(8 + 4 * h2, 12 + 4 * h2):
                    w_ = wop.tile([P, 2048], F16, tag="wop", bufs=8,
                                  name=f"wo_{ht}")
                    nc.sync.dma_start(w_[:], wo_d[:, ht, :])
                    wo_sb[ht] = w_

            # all 8 PSUM banks accumulate [m 0/1] x [Dc 0..3]
            ops_tags = ["mm", "mm", "mm", "aux", "acc", "acc", "acc", "acc"]
            ops_bufs = {"mm": 3, "aux": 1, "acc": 4}
            ops = []
            for m in range(2):
                for Dc in range(NJ):
                    tg = ops_tags[m * NJ + Dc]
                    ops.append(psp.tile([P, 512], F32, tag=tg,
                                        bufs=ops_bufs[tg], name=f"ops{m}_{Dc}"))
            o_sb = [tmp.tile([P, NJ, 512], F16, tag="osb", bufs=2,
                             name=f"osb_{m}") for m in range(2)]
            for ht in range(NT - 1):
                at_t = ATg[ht // 8]
                w_full = wo_sb[ht]
                for Dc in range(NJ):
                    for m in range(2):
                        mm = nc.tensor.matmul(
                            ops[m * NJ + Dc][:],
                            at_t[:, ht % 8, 128 * m:128 * m + 128],
                            w_full[:, 512 * Dc:512 * Dc + 512],
                            start=(ht == 0), stop=False)
                        if ht == 0:
                            # keep o-proj out of the PE stream until h1's
                            # attention logits are all issued: the scheduler
                            # otherwise slots these (collective-gated) ahead
                            # of ready attention work and stalls the PE; the
                            # trailing accumulations interleave fine
                            add_dep_helper(mm.ins, st_anchor[1],
                                           reason="oproj after attn1")
            # final contraction step m-major: m=0's output DMA leaves while
            # m=1's accumulators finish; copies alternate Act/DVE
            for m in range(2):
                for Dc in range(NJ):
                    nc.tensor.matmul(
                        ops[m * NJ + Dc][:],
                        ATg[1][:, 7, 128 * m:128 * m + 128],
                        wo_sb[NT - 1][:, 512 * Dc:512 * Dc + 512],
                        start=False, stop=True)
                    if Dc % 2 == 0:
                        nc.vector.tensor_copy(o_sb[m][:, Dc, :],
                                              ops[m * NJ + Dc][:])
                    else:
                        nc.scalar.activation(o_sb[m][:, Dc, :],
                                             ops[m * NJ + Dc][:], AF.Copy)
                nc.sync.dma_start(out_d[128 * m:128 * m + 128, :], o_sb[m][:])

    nc.compile()
    _dedupe_act_table_loads(nc)
    return nc


def _dedupe_act_table_loads(nc):
    """Bacc assigns Exp->exp_and_others and Ln->natural_log, inserting a
    ~2.7us table load at every Exp<->Ln alternation. All activation funcs
    this kernel uses (Exp, Ln, Square) live in the natural_log_exp_and_others
    set, so keep one load of that set and drop the rest."""
    from concourse.hw_specs import get_activation_tables
    tabs = list(get_activation_tables(nc.m.arch).items())
    nl_exp = next(i for i, (nm, funcs) in enumerate(tabs)
                  if nm == "natural_log_exp_and_others")
    used = {ins.func for bb in nc.main_func.blocks for ins in bb.instructions
            if isinstance(ins, mybir.InstActivation)}
    assert used <= tabs[nl_exp][1], f"funcs {used} not all in natural_log_exp"
    first = True
    for bb in nc.main_func.blocks:
        keep = []
        for ins in bb.instructions:
            if isinstance(ins, mybir.InstLoadActFuncSet):
                assert ins.sync_info is None or (
                    not ins.sync_info.on_wait and not ins.sync_info.on_update)
                if first:
                    ins.act_func_set_id = nl_exp
                    keep.append(ins)
                    first = False
                continue
            keep.append(ins)
        bb.instructions[:] = keep


def _host_prep(hidden_BTD, cos_BTK, sin_BTK, segment_ids_BT, position_ids_BT,
               wq, wk, wv, wo, q_norm_w, k_norm_w):
    hidden = np.ascontiguousarray(np.asarray(hidden_BTD, dtype=np.float32)[0])
    cos = np.asarray(cos_BTK, dtype=np.float32)[0]
    sin = np.asarray(sin_BTK, dtype=np.float32)[0]
    seg = np.asarray(segment_ids_BT)[0]
    pos = np.asarray(position_ids_BT)[0]
    wq = np.asarray(wq, dtype=np.float32)
    wk = np.asarray(wk, dtype=np.float32)
    wv = np.asarray(wv, dtype=np.float32)
    wo = np.asarray(wo, dtype=np.float32)
    q_norm_w = np.asarray(q_norm_w, dtype=np.float32)
    k_norm_w = np.asarray(k_norm_w, dtype=np.float32)

    assert np.array_equal(pos, np.arange(T, dtype=pos.dtype)), \
        "kernel assumes position_ids == arange"
    assert np.all(np.diff(seg) >= 0), "kernel assumes sorted segment ids"

    hT = np.ascontiguousarray(hidden.T.astype(np.float16))
    sqrtS = np.float32(np.sqrt(SCALE))
    signv = np.where(np.arange(HD) < HD // 2, -1.0, 1.0).astype(np.float32)
    shuf = (np.arange(HD) + HD // 2) % HD

    cosw = (cos.T * sqrtS).astype(np.float32)
    sinw = (sin.T * signv[:, None] * sqrtS).astype(np.float32)
    sinswap = sinw[shuf]  # halves swapped: see rotate-half ops in _build_program
    tblq = np.ascontiguousarray(np.stack([cosw, sinswap]).astype(np.float16))
    unit_w = bool(np.all(q_norm_w == 1.0) and np.all(k_norm_w == 1.0))
    wqk = np.ascontiguousarray(np.stack([q_norm_w, k_norm_w], axis=1))

    # prepack wo into partition-major layout; block order matches the
    # o-proj ht-step order (all h0 head-blocks, then all h1)
    perm = [2 * i + h for h in range(2) for i in range(NCORES)]
    wo_p = wo.reshape(NT, P, 2048)[perm].transpose(1, 0, 2)
    wo_p = np.ascontiguousarray(wo_p.astype(np.float16))

    seg_end = np.searchsorted(seg, seg, side="right").astype(np.int64)
    iota = np.broadcast_to(np.arange(512, dtype=np.float16), (P, 512)).copy()
    # causal staircase: for a diagonal tile the valid region relative to the
    # slice start is always (col-offset >= partition)
    caus = (np.arange(512)[None, :] >= np.arange(P)[:, None]).astype(np.float16)
    segrel = np.zeros((P, NT, NJ), dtype=np.float16)
    for i in range(NT):
        for j in range(NJ):
            segrel[:, i, j] = seg_end[P * i:P * i + P] - 512.0 * j

    in_maps = []
    for c in range(NCORES):
        h0, h1 = 2 * c, 2 * c + 1
        g = c // 2
        wqg = np.concatenate([
            wq[:, h0 * 256: h0 * 256 + 128],
            wq[:, h1 * 256: h1 * 256 + 128],
            wq[:, h0 * 256 + 128: h0 * 256 + 256],
            wq[:, h1 * 256 + 128: h1 * 256 + 256],
        ], axis=1).astype(np.float16)
        wqg_p = np.ascontiguousarray(wqg.reshape(DT, P, 512).transpose(1, 0, 2))
        wkv = np.concatenate([
            wk[:, g * 128:(g + 1) * 128], wv[:, g * 128:(g + 1) * 128]],
            axis=1).astype(np.float16)
        wkv_p = np.ascontiguousarray(wkv.reshape(DT, P, 256).transpose(1, 0, 2))
        m = {
            "hT": hT, "wqg": wqg_p, "wkv": wkv_p, "wo": wo_p,
            "tblq": tblq, "iota": iota, "caus": caus, "segrel": segrel,
        }
        if not unit_w:
            m["wqk"] = wqk
        in_maps.append(m)
    return in_maps, seg_end, unit_w


def kernel(**inputs) -> np.ndarray:
    in_maps, seg_end, unit_w = _host_prep(**inputs)
    key = (_tile_flags(seg_end), unit_w)
    if key not in _program_cache:
        _program_cache[key] = _build_program(key)
    nc = _program_cache[key]
    res = run_bass_kernel_spmd(nc, in_maps, list(range(NCORES)))
    out = np.concatenate([res.results[c]["out"] for c in range(NCORES)], axis=0)
    return out[None].astype(np.float32)
